# revision 17
# baseline (speedup 1.0000x reference)
"""Trainium2 Bass kernel for the per-sample dynamic-depthwise-conv block.

Computation (per sample b):
    att  = sigmoid(lrelu(v @ ca_w1.T) @ ca_w2.T)            # [b, 64]
    kern = (lrelu(v @ k_w1.T) @ k_w2.T).reshape(b*64,1,3,3) # per-(b,c) 3x3
    y    = lrelu(depthwise3x3(x0 * att, kern))
    out  = conv1x1(y, conv_w) + conv_b

Strategy: data-parallel over batch across 8 cores (4 samples/core).  On each
core, samples are processed in 2 "pairs"; a pair's 2x64 channels fill the 128
SBUF partitions.  The depthwise conv runs on the TensorEngine as 9
PSUM-accumulated matmuls with diagonal lhsT weights (attention folded in),
streaming shifted views of a zero-padded bf16 image tile.  Leaky-relu runs on
the Scalar engine (Prelu), and the final 1x1 conv is one matmul per chunk with
a block-diagonal [conv_w.T, conv_w.T] lhsT; bias is added by the Vector engine
during the PSUM->SBUF copy.
"""

import sys

if "/opt/trn_rl_repo" not in sys.path:
    sys.path.append("/opt/trn_rl_repo")

import numpy as np
import ml_dtypes

B, C, H, W = 32, 64, 128, 128
KK = 3
RED = 8
N_CORES = 8
BPC = B // N_CORES          # samples per core (4)
PAIRS = BPC // 2            # sample pairs per core (2)
HP, WP = H + 2, W + 2       # zero-padded image dims (130)
ROWS_PER_CHUNK = 4          # output rows per matmul chunk -> N = 4*128 = 512
NCHUNK = H // ROWS_PER_CHUNK

_CACHE = {}


def _build(repeat=1, n_taps=9):
    import concourse.bass as bass  # noqa: F401
    from concourse import bacc, tile, mybir

    f32 = mybir.dt.float32
    bf16 = mybir.dt.bfloat16
    AF = mybir.ActivationFunctionType

    nc = bacc.Bacc(None, target_bir_lowering=False, debug=False)

    x = nc.dram_tensor("x", [BPC, C, H, W], bf16, kind="ExternalInput")
    vt = nc.dram_tensor("vt", [C, BPC], f32, kind="ExternalInput")
    caw1t = nc.dram_tensor("caw1t", [C, RED], f32, kind="ExternalInput")
    caw2t = nc.dram_tensor("caw2t", [RED, C], f32, kind="ExternalInput")
    kw1t = nc.dram_tensor("kw1t", [C, C], f32, kind="ExternalInput")
    kw2t = nc.dram_tensor("kw2t", [C, C * KK * KK], f32, kind="ExternalInput")
    convt = nc.dram_tensor("convt", [128, 128], bf16, kind="ExternalInput")
    bcol = nc.dram_tensor("bcol", [128, 1], f32, kind="ExternalInput")
    eye = nc.dram_tensor("eye", [128, 128], f32, kind="ExternalInput")
    eyebf = nc.dram_tensor("eyebf", [128, 128], bf16, kind="ExternalInput")
    out = nc.dram_tensor("out", [BPC, C, H, W], bf16, kind="ExternalOutput")

    NK = C * KK * KK  # 576

    with tile.TileContext(nc) as tc:
        with (
            tc.tile_pool(name="consts", bufs=1) as consts,
            tc.tile_pool(name="stage", bufs=1) as stage,
            tc.tile_pool(name="diags", bufs=1) as diags,
            tc.tile_pool(name="xin", bufs=2) as xin,
            tc.tile_pool(name="parts", bufs=2) as parts,
            tc.tile_pool(name="ys", bufs=5) as ys,
            tc.tile_pool(name="os", bufs=6) as osb,
            tc.tile_pool(name="psA", bufs=2, space="PSUM") as psA,
            tc.tile_pool(name="psB", bufs=4, space="PSUM") as psB,
        ):
            # ---- constants into SBUF ----
            vt_sb = consts.tile([C, BPC], f32)
            caw1t_sb = consts.tile([C, RED], f32)
            caw2t_sb = consts.tile([RED, C], f32)
            kw1t_sb = consts.tile([C, C], f32)
            kw2t_sb = consts.tile([C, NK], f32)
            convt_sb = consts.tile([128, 128], bf16)
            bcol_sb = consts.tile([128, 1], f32)
            eye_sb = consts.tile([128, 128], f32)
            eyebf_sb = consts.tile([128, 128], bf16)
            for t, d in (
                (vt_sb, vt), (caw1t_sb, caw1t), (caw2t_sb, caw2t),
                (kw1t_sb, kw1t), (kw2t_sb, kw2t), (convt_sb, convt),
                (bcol_sb, bcol), (eye_sb, eye), (eyebf_sb, eyebf),
            ):
                nc.sync.dma_start(out=t[:], in_=d.ap())

            import contextlib
            rep_ctx = (tc.For_i(0, repeat, 1) if repeat > 1
                       else contextlib.nullcontext())
            with rep_ctx:
                _body(nc, tc, mybir, AF, f32, bf16,
                      consts, stage, diags, xin, parts, ys, osb,
                      psA, psB, psA,
                      vt_sb, caw1t_sb, caw2t_sb, kw1t_sb, kw2t_sb,
                      convt_sb, bcol_sb, eye_sb, eyebf_sb, x, out, n_taps)

    nc.compile()
    return nc


def _body(nc, tc, mybir, AF, f32, bf16,
          consts, stage, diags, xin, parts, ys, osb, psA, psB, psS,
          vt_sb, caw1t_sb, caw2t_sb, kw1t_sb, kw2t_sb,
          convt_sb, bcol_sb, eye_sb, eyebf_sb, x, out, n_taps=9):
    NK = C * KK * KK
    if True:
        if True:

            # ---- tiny MLP stage: attention + generated kernels ----
            # h1T = lrelu(ca_w1 @ v.T)                       [8, BPC]
            ps_h1 = psA.tile([RED, BPC], f32, tag="pa", name="ps_h1")
            nc.tensor.matmul(ps_h1[:], lhsT=caw1t_sb[:], rhs=vt_sb[:],
                             start=True, stop=True)
            h1t_sb = stage.tile([RED, BPC], f32)
            nc.scalar.activation(h1t_sb[:], ps_h1[:], AF.Prelu, alpha=0.1)

            # attT = sigmoid(ca_w2 @ h1)                     [64, BPC]
            ps_att = psA.tile([C, BPC], f32, tag="pa", name="ps_att")
            nc.tensor.matmul(ps_att[:], lhsT=caw2t_sb[:], rhs=h1t_sb[:],
                             start=True, stop=True)
            att_sb = stage.tile([C, BPC], f32)
            nc.scalar.activation(att_sb[:], ps_att[:], AF.Sigmoid)

            # h2T = lrelu(k_w1 @ v.T)                        [64, BPC]
            ps_h2 = psA.tile([C, BPC], f32, tag="pa", name="ps_h2")
            nc.tensor.matmul(ps_h2[:], lhsT=kw1t_sb[:], rhs=vt_sb[:],
                             start=True, stop=True)
            h2t_sb = stage.tile([C, BPC], f32)
            nc.scalar.activation(h2t_sb[:], ps_h2[:], AF.Prelu, alpha=0.1)

            # kern = h2 @ k_w2.T                             [BPC, 576]
            ps_k = psA.tile([BPC, NK], f32, tag="pa", name="ps_k")
            nc.tensor.matmul(ps_k[:, 0:512], lhsT=h2t_sb[:],
                             rhs=kw2t_sb[:, 0:512], start=True, stop=True)
            nc.tensor.matmul(ps_k[:, 512:NK], lhsT=h2t_sb[:],
                             rhs=kw2t_sb[:, 512:NK], start=True, stop=True)
            kern_sb = stage.tile([BPC, NK], f32)
            nc.scalar.activation(kern_sb[:], ps_k[:], AF.Copy)

            # ---- gather per-pair tap scalars: p = s*64 + c on partitions ----
            dtap_sb = stage.tile([128, PAIRS, KK * KK], f32)
            attpp_sb = stage.tile([128, PAIRS], f32)
            for pr in range(PAIRS):
                for s in range(2):
                    b = pr * 2 + s
                    # src [1, 64, 9] and dest [64, 1, 9] match in flattened
                    # element order (dma_start maps by flat AP order).
                    src = kern_sb[b:b + 1, :].rearrange(
                        "o (c t) -> o c t", c=C)
                    nc.sync.dma_start(
                        out=dtap_sb[C * s:C * (s + 1), pr:pr + 1, :], in_=src)
                    nc.sync.dma_start(
                        out=attpp_sb[C * s:C * (s + 1), pr:pr + 1],
                        in_=att_sb[:, b:b + 1])

            # tap split: even-dj taps run on the Vector engine (bf16 2x
            # mode needs 4-byte alignment); the rest run on the PE as
            # diagonal matmuls.
            DVE_TAPS = [(0, 0), (0, 2), (1, 0), (1, 2)]
            PE_TAPS = [(0, 1), (1, 1), (2, 1), (2, 0), (2, 2)]

            # d[p, t] = att[p] * kern[p, t]; diag tiles = eye * d[:, t]
            diag = [{} for _ in range(PAIRS)]
            dcols = []
            for pr in range(PAIRS):
                d_pr = stage.tile([128, KK * KK], f32, tag=f"d{pr}")
                nc.vector.tensor_scalar_mul(
                    d_pr[:], dtap_sb[:, pr, :], attpp_sb[:, pr:pr + 1])
                dcols.append(d_pr)  # noqa
                for (di, dj) in PE_TAPS:
                    t = di * KK + dj
                    dg = diags.tile([128, 128], bf16, tag=f"diag{pr}_{t}")
                    nc.vector.tensor_scalar_mul(
                        dg[:], eye_sb[:], d_pr[:, t:t + 1])
                    diag[pr][(di, dj)] = dg

            # ---- main loop ----
            xv = x.ap().rearrange("(pr s) c h w -> pr (s c) h w", pr=PAIRS)
            ov = out.ap().rearrange("(pr s) c h w -> pr (s c) h w", pr=PAIRS)

            GRP = 4  # chunks per weight-stationary group (psA banks)
            for pr in range(PAIRS):
                xt = xin.tile([128, HP, WP], bf16, tag="xt")
                # zero the one-pixel border; the interior is fully overwritten
                nc.vector.memset(xt[:, 0, :], 0.0)
                nc.vector.memset(xt[:, HP - 1, :], 0.0)
                nc.vector.memset(xt[:, 1:HP - 1, 0], 0.0)
                nc.vector.memset(xt[:, 1:HP - 1, WP - 1], 0.0)
                # split the 4 MiB load across DMA queues
                nsplit = 8
                rstep = H // nsplit
                for k in range(nsplit):
                    r0 = k * rstep
                    nc.sync.dma_start(
                        out=xt[:, 1 + r0:1 + r0 + rstep, 1:WP - 1],
                        in_=xv[pr, :, r0:r0 + rstep, :])

                BR = 2 * GRP * ROWS_PER_CHUNK  # partial rows per block (32)
                parts_of = {}
                for gb in range(0, NCHUNK, 2 * GRP):
                    r0 = (gb // (2 * GRP)) * BR
                    part = parts.tile([128, BR, W], bf16, tag="part",
                                      bufs=3, name=f"part{gb}")
                    parts_of[gb] = (part, 0)
                    parts_of[gb + GRP] = (part, GRP * ROWS_PER_CHUNK)
                    # Vector engine: accumulate the even-aligned taps for
                    # this 32-row block of the pair image (bf16 2x/4x modes)
                    # ts (4x mode) + tt-add (2x mode) beat one fused
                    # scalar_tensor_tensor (1x-only uop) on the DVE
                    for n, (di, dj) in enumerate(DVE_TAPS):
                        t = di * KK + dj
                        blk = part[:, :, :]
                        xin_v = xt[:, r0 + di:r0 + di + BR, dj:dj + W]
                        if n == 0:
                            nc.vector.tensor_scalar_mul(
                                blk, xin_v, dcols[pr][:, t:t + 1])
                        else:
                            tmp = parts.tile([128, BR, W], bf16, tag="tmp",
                                             name=f"tmp{gb}_{n}")
                            nc.vector.tensor_scalar_mul(
                                tmp[:], xin_v, dcols[pr][:, t:t + 1])
                            nc.vector.tensor_add(blk, blk, tmp[:])

                for g in range(0, NCHUNK, GRP):
                    part, roff = parts_of[g]
                    NW = ROWS_PER_CHUNK * W  # 512
                    pas = [psA.tile([128, 2 * NW], f32,
                                    tag="pa", name=f"pa{g}_{h}")
                           for h in range(GRP // 2)]
                    # PE: remaining taps as diagonal matmuls (weight loaded
                    # once per GRP), then the DVE partial via identity matmul
                    for t, (di, dj) in enumerate(PE_TAPS):
                        for c in range(GRP):
                            i0 = (g + c) * ROWS_PER_CHUNK
                            nc.tensor.matmul(
                                pas[c // 2][:, (c % 2) * NW:
                                            (c % 2) * NW + NW],
                                lhsT=diag[pr][(di, dj)][:],
                                rhs=xt[:, i0 + di:i0 + di + ROWS_PER_CHUNK,
                                       dj:dj + W],
                                start=(t == 0), stop=False,
                                skip_group_check=True)
                    for c in range(GRP):
                        rc = roff + c * ROWS_PER_CHUNK
                        nc.tensor.matmul(
                            pas[c // 2][:, (c % 2) * NW:(c % 2) * NW + NW],
                            lhsT=eyebf_sb[:],
                            rhs=part[:, rc:rc + ROWS_PER_CHUNK, :],
                            start=False, stop=True, skip_group_check=True)
                    for h in range(GRP // 2):
                        i0 = (g + 2 * h) * ROWS_PER_CHUNK
                        yt = ys.tile([128, 2 * NW], bf16, tag="yt")
                        nc.scalar.activation(yt[:], pas[h][:], AF.Prelu,
                                             alpha=0.1)
                        for c2 in range(2):
                            pb = psB.tile([128, NW], f32, tag="pb")
                            nc.tensor.matmul(
                                pb[:], lhsT=convt_sb[:],
                                rhs=yt[:, c2 * NW:c2 * NW + NW],
                                start=True, stop=True)
                            ot = osb.tile([128, NW], bf16, tag="ot")
                            nc.scalar.activation(ot[:], pb[:], AF.Identity,
                                                 bias=bcol_sb[:, 0:1])
                            j0 = i0 + c2 * ROWS_PER_CHUNK
                            nc.sync.dma_start(
                                out=ov[pr, :, j0:j0 + ROWS_PER_CHUNK, :],
                                in_=ot[:].rearrange("p (r w) -> p r w",
                                                    r=ROWS_PER_CHUNK))


def get_nc(repeat=1, n_taps=9):
    key = ("nc", repeat, n_taps)
    if key not in _CACHE:
        _CACHE[key] = _build(repeat, n_taps)
    return _CACHE[key]


def make_in_maps(x0, v, ca_w1, ca_w2, k_w1, k_w2, conv_w, conv_b):
    bf = ml_dtypes.bfloat16
    caw1t = np.ascontiguousarray(ca_w1.T, dtype=np.float32)
    caw2t = np.ascontiguousarray(ca_w2.T, dtype=np.float32)
    kw1t = np.ascontiguousarray(k_w1.T, dtype=np.float32)
    kw2t = np.ascontiguousarray(k_w2.T, dtype=np.float32)
    convt = np.zeros((128, 128), dtype=bf)
    cwt = conv_w.T.astype(bf)
    convt[0:64, 0:64] = cwt
    convt[64:128, 64:128] = cwt
    bcol = np.tile(conv_b.astype(np.float32), 2)[:, None].copy()
    eye = np.eye(128, dtype=np.float32)
    eyebf = np.eye(128, dtype=bf)
    in_maps = []
    for k in range(N_CORES):
        sl = slice(k * BPC, (k + 1) * BPC)
        in_maps.append({
            "x": np.ascontiguousarray(x0[sl]).astype(bf),
            "vt": np.ascontiguousarray(v[sl].T, dtype=np.float32),
            "caw1t": caw1t, "caw2t": caw2t, "kw1t": kw1t, "kw2t": kw2t,
            "convt": convt, "bcol": bcol, "eye": eye, "eyebf": eyebf,
        })
    return in_maps


def kernel(x0, v, ca_w1, ca_w2, k_w1, k_w2, conv_w, conv_b):
    from concourse.bass_utils import run_bass_kernel_spmd

    nc = get_nc()
    in_maps = make_in_maps(x0, v, ca_w1, ca_w2, k_w1, k_w2, conv_w, conv_b)
    res = run_bass_kernel_spmd(nc, in_maps, list(range(N_CORES)))
    return np.concatenate([res.results[i]["out"] for i in range(N_CORES)],
                          axis=0).astype(np.float32)


# revision 18
# speedup vs baseline: 1.0104x; 1.0104x over previous
"""Trainium2 Bass kernel for the per-sample dynamic-depthwise-conv block.

Computation (per sample b):
    att  = sigmoid(lrelu(v @ ca_w1.T) @ ca_w2.T)            # [b, 64]
    kern = (lrelu(v @ k_w1.T) @ k_w2.T).reshape(b*64,1,3,3) # per-(b,c) 3x3
    y    = lrelu(depthwise3x3(x0 * att, kern))
    out  = conv1x1(y, conv_w) + conv_b

Strategy: data-parallel over batch across 8 cores (4 samples/core).  On each
core, samples are processed in 2 "pairs"; a pair's 2x64 channels fill the 128
SBUF partitions.  The attention gate is folded into the generated tap weights
(dw(att*x) == att*dw(x) per channel), so x0 streams straight from HBM as bf16
into a zero-padded [128, 130, 130] SBUF tile.  The 9 depthwise taps are split
across engines:
  - 5 taps (the odd-alignment ones) run on the TensorEngine as PSUM-
    accumulated matmuls with diagonal bf16 lhsT weights against shifted views
    of the padded tile (weight-stationary over groups of 4 chunks);
  - 4 even-aligned taps run on the Vector engine in bf16 (tensor_scalar at 4x
    mode + tensor_tensor add at 2x mode over 32-row blocks) and are injected
    into the same PSUM accumulation via one identity matmul per chunk.
Leaky-relu runs on the Scalar engine (Prelu, PSUM->SBUF, bf16); the final 1x1
conv is one matmul per 512-column chunk with a block-diagonal
[conv_w.T, conv_w.T] lhsT; conv bias rides the Scalar engine's Identity
activation during the PSUM->SBUF copy; outputs leave as bf16 and are widened
to fp32 on the host.  The tiny channel-attention/kernel-generating MLPs run
once per core on the PE/ACT engines at fp32.
"""

import sys

if "/opt/trn_rl_repo" not in sys.path:
    sys.path.append("/opt/trn_rl_repo")

import numpy as np
import ml_dtypes

B, C, H, W = 32, 64, 128, 128
KK = 3
RED = 8
N_CORES = 8
BPC = B // N_CORES          # samples per core (4)
PAIRS = BPC // 2            # sample pairs per core (2)
HP, WP = H + 2, W + 2       # zero-padded image dims (130)
ROWS_PER_CHUNK = 4          # output rows per matmul chunk -> N = 4*128 = 512
NCHUNK = H // ROWS_PER_CHUNK

_CACHE = {}


def _build(repeat=1, n_taps=9):
    import concourse.bass as bass  # noqa: F401
    from concourse import bacc, tile, mybir

    f32 = mybir.dt.float32
    bf16 = mybir.dt.bfloat16
    AF = mybir.ActivationFunctionType

    nc = bacc.Bacc(None, target_bir_lowering=False, debug=False)

    x = nc.dram_tensor("x", [BPC, C, H, W], bf16, kind="ExternalInput")
    vt = nc.dram_tensor("vt", [C, BPC], f32, kind="ExternalInput")
    caw1t = nc.dram_tensor("caw1t", [C, RED], f32, kind="ExternalInput")
    caw2t = nc.dram_tensor("caw2t", [RED, C], f32, kind="ExternalInput")
    kw1t = nc.dram_tensor("kw1t", [C, C], f32, kind="ExternalInput")
    kw2t = nc.dram_tensor("kw2t", [C, C * KK * KK], f32, kind="ExternalInput")
    convt = nc.dram_tensor("convt", [128, 128], bf16, kind="ExternalInput")
    bcol = nc.dram_tensor("bcol", [128, 1], f32, kind="ExternalInput")
    eye = nc.dram_tensor("eye", [128, 128], f32, kind="ExternalInput")
    eyebf = nc.dram_tensor("eyebf", [128, 128], bf16, kind="ExternalInput")
    out = nc.dram_tensor("out", [BPC, C, H, W], bf16, kind="ExternalOutput")

    NK = C * KK * KK  # 576

    with tile.TileContext(nc) as tc:
        with (
            tc.tile_pool(name="consts", bufs=1) as consts,
            tc.tile_pool(name="stage", bufs=1) as stage,
            tc.tile_pool(name="diags", bufs=1) as diags,
            tc.tile_pool(name="xin", bufs=2) as xin,
            tc.tile_pool(name="parts", bufs=2) as parts,
            tc.tile_pool(name="ys", bufs=5) as ys,
            tc.tile_pool(name="os", bufs=6) as osb,
            tc.tile_pool(name="psA", bufs=2, space="PSUM") as psA,
            tc.tile_pool(name="psB", bufs=4, space="PSUM") as psB,
        ):
            # ---- constants into SBUF ----
            vt_sb = consts.tile([C, BPC], f32)
            caw1t_sb = consts.tile([C, RED], f32)
            caw2t_sb = consts.tile([RED, C], f32)
            kw1t_sb = consts.tile([C, C], f32)
            kw2t_sb = consts.tile([C, NK], f32)
            convt_sb = consts.tile([128, 128], bf16)
            bcol_sb = consts.tile([128, 1], f32)
            eye_sb = consts.tile([128, 128], f32)
            eyebf_sb = consts.tile([128, 128], bf16)
            for t, d in (
                (vt_sb, vt), (caw1t_sb, caw1t), (caw2t_sb, caw2t),
                (kw1t_sb, kw1t), (kw2t_sb, kw2t), (convt_sb, convt),
                (bcol_sb, bcol), (eye_sb, eye), (eyebf_sb, eyebf),
            ):
                nc.sync.dma_start(out=t[:], in_=d.ap())

            import contextlib
            rep_ctx = (tc.For_i(0, repeat, 1) if repeat > 1
                       else contextlib.nullcontext())
            with rep_ctx:
                _body(nc, tc, mybir, AF, f32, bf16,
                      consts, stage, diags, xin, parts, ys, osb,
                      psA, psB, psA,
                      vt_sb, caw1t_sb, caw2t_sb, kw1t_sb, kw2t_sb,
                      convt_sb, bcol_sb, eye_sb, eyebf_sb, x, out, n_taps)

    nc.compile()
    return nc


def _body(nc, tc, mybir, AF, f32, bf16,
          consts, stage, diags, xin, parts, ys, osb, psA, psB, psS,
          vt_sb, caw1t_sb, caw2t_sb, kw1t_sb, kw2t_sb,
          convt_sb, bcol_sb, eye_sb, eyebf_sb, x, out, n_taps=9):
    NK = C * KK * KK
    if True:
        if True:

            # ---- tiny MLP stage: attention + generated kernels ----
            # h1T = lrelu(ca_w1 @ v.T)                       [8, BPC]
            ps_h1 = psA.tile([RED, BPC], f32, tag="pa", name="ps_h1")
            nc.tensor.matmul(ps_h1[:], lhsT=caw1t_sb[:], rhs=vt_sb[:],
                             start=True, stop=True)
            h1t_sb = stage.tile([RED, BPC], f32)
            nc.scalar.activation(h1t_sb[:], ps_h1[:], AF.Prelu, alpha=0.1)

            # attT = sigmoid(ca_w2 @ h1)                     [64, BPC]
            ps_att = psA.tile([C, BPC], f32, tag="pa", name="ps_att")
            nc.tensor.matmul(ps_att[:], lhsT=caw2t_sb[:], rhs=h1t_sb[:],
                             start=True, stop=True)
            att_sb = stage.tile([C, BPC], f32)
            nc.scalar.activation(att_sb[:], ps_att[:], AF.Sigmoid)

            # h2T = lrelu(k_w1 @ v.T)                        [64, BPC]
            ps_h2 = psA.tile([C, BPC], f32, tag="pa", name="ps_h2")
            nc.tensor.matmul(ps_h2[:], lhsT=kw1t_sb[:], rhs=vt_sb[:],
                             start=True, stop=True)
            h2t_sb = stage.tile([C, BPC], f32)
            nc.scalar.activation(h2t_sb[:], ps_h2[:], AF.Prelu, alpha=0.1)

            # kern = h2 @ k_w2.T                             [BPC, 576]
            ps_k = psA.tile([BPC, NK], f32, tag="pa", name="ps_k")
            nc.tensor.matmul(ps_k[:, 0:512], lhsT=h2t_sb[:],
                             rhs=kw2t_sb[:, 0:512], start=True, stop=True)
            nc.tensor.matmul(ps_k[:, 512:NK], lhsT=h2t_sb[:],
                             rhs=kw2t_sb[:, 512:NK], start=True, stop=True)
            kern_sb = stage.tile([BPC, NK], f32)
            nc.scalar.activation(kern_sb[:], ps_k[:], AF.Copy)

            # ---- gather per-pair tap scalars: p = s*64 + c on partitions ----
            dtap_sb = stage.tile([128, PAIRS, KK * KK], f32)
            attpp_sb = stage.tile([128, PAIRS], f32)
            for pr in range(PAIRS):
                for s in range(2):
                    b = pr * 2 + s
                    # src [1, 64, 9] and dest [64, 1, 9] match in flattened
                    # element order (dma_start maps by flat AP order).
                    src = kern_sb[b:b + 1, :].rearrange(
                        "o (c t) -> o c t", c=C)
                    nc.sync.dma_start(
                        out=dtap_sb[C * s:C * (s + 1), pr:pr + 1, :], in_=src)
                    nc.sync.dma_start(
                        out=attpp_sb[C * s:C * (s + 1), pr:pr + 1],
                        in_=att_sb[:, b:b + 1])

            # tap split: even-dj taps run on the Vector engine (bf16 2x
            # mode needs 4-byte alignment); the rest run on the PE as
            # diagonal matmuls.
            DVE_TAPS = [(0, 0), (0, 2), (1, 0), (1, 2)]
            PE_TAPS = [(0, 1), (1, 1), (2, 1), (2, 0), (2, 2)]

            # d[p, t] = att[p] * kern[p, t]; diag tiles = eye * d[:, t]
            diag = [{} for _ in range(PAIRS)]
            dcols = []
            for pr in range(PAIRS):
                d_pr = stage.tile([128, KK * KK], f32, tag=f"d{pr}")
                nc.vector.tensor_scalar_mul(
                    d_pr[:], dtap_sb[:, pr, :], attpp_sb[:, pr:pr + 1])
                dcols.append(d_pr)  # noqa
                for (di, dj) in PE_TAPS:
                    t = di * KK + dj
                    dg = diags.tile([128, 128], bf16, tag=f"diag{pr}_{t}")
                    nc.vector.tensor_scalar_mul(
                        dg[:], eye_sb[:], d_pr[:, t:t + 1])
                    diag[pr][(di, dj)] = dg

            # ---- main loop ----
            xv = x.ap().rearrange("(pr s) c h w -> pr (s c) h w", pr=PAIRS)
            ov = out.ap().rearrange("(pr s) c h w -> pr (s c) h w", pr=PAIRS)

            GRP = 4  # chunks per weight-stationary group (psA banks)
            for pr in range(PAIRS):
                xt = xin.tile([128, HP, WP], bf16, tag="xt")
                # zero the one-pixel border; the interior is fully overwritten
                nc.vector.memset(xt[:, 0, :], 0.0)
                nc.vector.memset(xt[:, HP - 1, :], 0.0)
                nc.vector.memset(xt[:, 1:HP - 1, 0], 0.0)
                nc.vector.memset(xt[:, 1:HP - 1, WP - 1], 0.0)
                # split the 4 MiB load across DMA queues
                nsplit = 8
                rstep = H // nsplit
                for k in range(nsplit):
                    r0 = k * rstep
                    nc.sync.dma_start(
                        out=xt[:, 1 + r0:1 + r0 + rstep, 1:WP - 1],
                        in_=xv[pr, :, r0:r0 + rstep, :])

                BR = 2 * GRP * ROWS_PER_CHUNK  # partial rows per block (32)
                parts_of = {}
                for gb in range(0, NCHUNK, 2 * GRP):
                    r0 = (gb // (2 * GRP)) * BR
                    part = parts.tile([128, BR, W], bf16, tag="part",
                                      bufs=3, name=f"part{gb}")
                    parts_of[gb] = (part, 0)
                    parts_of[gb + GRP] = (part, GRP * ROWS_PER_CHUNK)
                    # Vector engine: accumulate the even-aligned taps for
                    # this 32-row block of the pair image (bf16 2x/4x modes)
                    # ts (4x mode) + tt-add (2x mode) beat one fused
                    # scalar_tensor_tensor (1x-only uop) on the DVE
                    for n, (di, dj) in enumerate(DVE_TAPS):
                        t = di * KK + dj
                        blk = part[:, :, :]
                        xin_v = xt[:, r0 + di:r0 + di + BR, dj:dj + W]
                        if n == 0:
                            nc.vector.tensor_scalar_mul(
                                blk, xin_v, dcols[pr][:, t:t + 1])
                        else:
                            tmp = parts.tile([128, BR, W], bf16, tag="tmp",
                                             name=f"tmp{gb}_{n}")
                            nc.vector.tensor_scalar_mul(
                                tmp[:], xin_v, dcols[pr][:, t:t + 1])
                            nc.vector.tensor_add(blk, blk, tmp[:])

                for g in range(0, NCHUNK, GRP):
                    part, roff = parts_of[g]
                    NW = ROWS_PER_CHUNK * W  # 512
                    pas = [psA.tile([128, 2 * NW], f32,
                                    tag="pa", name=f"pa{g}_{h}")
                           for h in range(GRP // 2)]
                    # PE: remaining taps as diagonal matmuls (weight loaded
                    # once per GRP), then the DVE partial via identity matmul
                    for t, (di, dj) in enumerate(PE_TAPS):
                        for c in range(GRP):
                            i0 = (g + c) * ROWS_PER_CHUNK
                            nc.tensor.matmul(
                                pas[c // 2][:, (c % 2) * NW:
                                            (c % 2) * NW + NW],
                                lhsT=diag[pr][(di, dj)][:],
                                rhs=xt[:, i0 + di:i0 + di + ROWS_PER_CHUNK,
                                       dj:dj + W],
                                start=(t == 0), stop=False,
                                skip_group_check=True)
                    for c in range(GRP):
                        rc = roff + c * ROWS_PER_CHUNK
                        nc.tensor.matmul(
                            pas[c // 2][:, (c % 2) * NW:(c % 2) * NW + NW],
                            lhsT=eyebf_sb[:],
                            rhs=part[:, rc:rc + ROWS_PER_CHUNK, :],
                            start=False, stop=True, skip_group_check=True)
                    for h in range(GRP // 2):
                        i0 = (g + 2 * h) * ROWS_PER_CHUNK
                        yt = ys.tile([128, 2 * NW], bf16, tag="yt")
                        nc.scalar.activation(yt[:], pas[h][:], AF.Prelu,
                                             alpha=0.1)
                        for c2 in range(2):
                            pb = psB.tile([128, NW], f32, tag="pb")
                            nc.tensor.matmul(
                                pb[:], lhsT=convt_sb[:],
                                rhs=yt[:, c2 * NW:c2 * NW + NW],
                                start=True, stop=True)
                            ot = osb.tile([128, NW], bf16, tag="ot")
                            nc.scalar.activation(ot[:], pb[:], AF.Identity,
                                                 bias=bcol_sb[:, 0:1])
                            j0 = i0 + c2 * ROWS_PER_CHUNK
                            nc.sync.dma_start(
                                out=ov[pr, :, j0:j0 + ROWS_PER_CHUNK, :],
                                in_=ot[:].rearrange("p (r w) -> p r w",
                                                    r=ROWS_PER_CHUNK))


def get_nc(repeat=1, n_taps=9):
    key = ("nc", repeat, n_taps)
    if key not in _CACHE:
        _CACHE[key] = _build(repeat, n_taps)
    return _CACHE[key]


def make_in_maps(x0, v, ca_w1, ca_w2, k_w1, k_w2, conv_w, conv_b):
    bf = ml_dtypes.bfloat16
    caw1t = np.ascontiguousarray(ca_w1.T, dtype=np.float32)
    caw2t = np.ascontiguousarray(ca_w2.T, dtype=np.float32)
    kw1t = np.ascontiguousarray(k_w1.T, dtype=np.float32)
    kw2t = np.ascontiguousarray(k_w2.T, dtype=np.float32)
    convt = np.zeros((128, 128), dtype=bf)
    cwt = conv_w.T.astype(bf)
    convt[0:64, 0:64] = cwt
    convt[64:128, 64:128] = cwt
    bcol = np.tile(conv_b.astype(np.float32), 2)[:, None].copy()
    eye = np.eye(128, dtype=np.float32)
    eyebf = np.eye(128, dtype=bf)
    in_maps = []
    for k in range(N_CORES):
        sl = slice(k * BPC, (k + 1) * BPC)
        in_maps.append({
            "x": np.ascontiguousarray(x0[sl]).astype(bf),
            "vt": np.ascontiguousarray(v[sl].T, dtype=np.float32),
            "caw1t": caw1t, "caw2t": caw2t, "kw1t": kw1t, "kw2t": kw2t,
            "convt": convt, "bcol": bcol, "eye": eye, "eyebf": eyebf,
        })
    return in_maps


def kernel(x0, v, ca_w1, ca_w2, k_w1, k_w2, conv_w, conv_b):
    from concourse.bass_utils import run_bass_kernel_spmd

    nc = get_nc()
    in_maps = make_in_maps(x0, v, ca_w1, ca_w2, k_w1, k_w2, conv_w, conv_b)
    res = run_bass_kernel_spmd(nc, in_maps, list(range(N_CORES)))
    return np.concatenate([res.results[i]["out"] for i in range(N_CORES)],
                          axis=0).astype(np.float32)


# revision 20
# speedup vs baseline: 1.0553x; 1.0444x over previous
"""Trainium2 Bass kernel for the per-sample dynamic-depthwise-conv block.

Computation (per sample b):
    att  = sigmoid(lrelu(v @ ca_w1.T) @ ca_w2.T)            # [b, 64]
    kern = (lrelu(v @ k_w1.T) @ k_w2.T).reshape(b*64,1,3,3) # per-(b,c) 3x3
    y    = lrelu(depthwise3x3(x0 * att, kern))
    out  = conv1x1(y, conv_w) + conv_b

Strategy: data-parallel over batch across 8 cores (4 samples/core).  On each
core, samples are processed in 2 "pairs"; a pair's 2x64 channels fill the 128
SBUF partitions.  The attention gate is folded into the generated tap weights
(dw(att*x) == att*dw(x) per channel), so x0 streams straight from HBM as bf16
into a zero-padded [128, 130, 130] SBUF tile.  The 9 depthwise taps are split
across engines:
  - 5 taps (the odd-alignment ones) run on the TensorEngine as PSUM-
    accumulated matmuls with diagonal bf16 lhsT weights against shifted views
    of the padded tile (weight-stationary over groups of 4 chunks);
  - 4 even-aligned taps run on the Vector engine in bf16 (tensor_scalar at 4x
    mode + tensor_tensor add at 2x mode over 32-row blocks) and are injected
    into the same PSUM accumulation via one identity matmul per chunk.
Leaky-relu runs on the Scalar engine (Prelu, PSUM->SBUF, bf16); the final 1x1
conv is one matmul per 512-column chunk with a block-diagonal
[conv_w.T, conv_w.T] lhsT; conv bias rides the Scalar engine's Identity
activation during the PSUM->SBUF copy; outputs leave as bf16 and are widened
to fp32 on the host.  The tiny channel-attention/kernel-generating MLPs run
once per core on the PE/ACT engines at fp32.
"""

import sys

if "/opt/trn_rl_repo" not in sys.path:
    sys.path.append("/opt/trn_rl_repo")

import numpy as np
import ml_dtypes

B, C, H, W = 32, 64, 128, 128
KK = 3
RED = 8
N_CORES = 8
BPC = B // N_CORES          # samples per core (4)
PAIRS = BPC // 2            # sample pairs per core (2)
HP, WP = H + 2, W + 2       # zero-padded image dims (130)
ROWS_PER_CHUNK = 4          # output rows per matmul chunk -> N = 4*128 = 512
NCHUNK = H // ROWS_PER_CHUNK

_CACHE = {}


def _build(repeat=1, n_taps=9):
    import concourse.bass as bass  # noqa: F401
    from concourse import bacc, tile, mybir

    f32 = mybir.dt.float32
    bf16 = mybir.dt.bfloat16
    AF = mybir.ActivationFunctionType

    nc = bacc.Bacc(None, target_bir_lowering=False, debug=False)

    x = nc.dram_tensor("x", [BPC, C, H, W], bf16, kind="ExternalInput")
    vt = nc.dram_tensor("vt", [C, BPC], f32, kind="ExternalInput")
    caw1t = nc.dram_tensor("caw1t", [C, RED], f32, kind="ExternalInput")
    caw2t = nc.dram_tensor("caw2t", [RED, C], f32, kind="ExternalInput")
    kw1t = nc.dram_tensor("kw1t", [C, C], f32, kind="ExternalInput")
    kw2t = nc.dram_tensor("kw2t", [C, C * KK * KK], f32, kind="ExternalInput")
    convt = nc.dram_tensor("convt", [128, 128], bf16, kind="ExternalInput")
    bcol = nc.dram_tensor("bcol", [128, 1], f32, kind="ExternalInput")
    eye = nc.dram_tensor("eye", [128, 128], f32, kind="ExternalInput")
    eyebf = nc.dram_tensor("eyebf", [128, 128], bf16, kind="ExternalInput")
    out = nc.dram_tensor("out", [BPC, C, H, W], bf16, kind="ExternalOutput")

    NK = C * KK * KK  # 576

    with tile.TileContext(nc) as tc:
        with (
            tc.tile_pool(name="consts", bufs=1) as consts,
            tc.tile_pool(name="stage", bufs=1) as stage,
            tc.tile_pool(name="diags", bufs=1) as diags,
            tc.tile_pool(name="xin", bufs=2) as xin,
            tc.tile_pool(name="parts", bufs=2) as parts,
            tc.tile_pool(name="ys", bufs=5) as ys,
            tc.tile_pool(name="os", bufs=6) as osb,
            tc.tile_pool(name="psA", bufs=3, space="PSUM") as psA,
            tc.tile_pool(name="psB", bufs=2, space="PSUM") as psB,
        ):
            # ---- constants into SBUF ----
            vt_sb = consts.tile([C, BPC], f32)
            caw1t_sb = consts.tile([C, RED], f32)
            caw2t_sb = consts.tile([RED, C], f32)
            kw1t_sb = consts.tile([C, C], f32)
            kw2t_sb = consts.tile([C, NK], f32)
            convt_sb = consts.tile([128, 128], bf16)
            bcol_sb = consts.tile([128, 1], f32)
            eye_sb = consts.tile([128, 128], f32)
            eyebf_sb = consts.tile([128, 128], bf16)
            for t, d in (
                (vt_sb, vt), (caw1t_sb, caw1t), (caw2t_sb, caw2t),
                (kw1t_sb, kw1t), (kw2t_sb, kw2t), (convt_sb, convt),
                (bcol_sb, bcol), (eye_sb, eye), (eyebf_sb, eyebf),
            ):
                nc.sync.dma_start(out=t[:], in_=d.ap())

            import contextlib
            rep_ctx = (tc.For_i(0, repeat, 1) if repeat > 1
                       else contextlib.nullcontext())
            with rep_ctx:
                _body(nc, tc, mybir, AF, f32, bf16,
                      consts, stage, diags, xin, parts, ys, osb,
                      psA, psB, psA,
                      vt_sb, caw1t_sb, caw2t_sb, kw1t_sb, kw2t_sb,
                      convt_sb, bcol_sb, eye_sb, eyebf_sb, x, out, n_taps)

    nc.compile()
    return nc


def _body(nc, tc, mybir, AF, f32, bf16,
          consts, stage, diags, xin, parts, ys, osb, psA, psB, psS,
          vt_sb, caw1t_sb, caw2t_sb, kw1t_sb, kw2t_sb,
          convt_sb, bcol_sb, eye_sb, eyebf_sb, x, out, n_taps=9):
    NK = C * KK * KK
    if True:
        if True:

            # ---- tiny MLP stage: attention + generated kernels ----
            # h1T = lrelu(ca_w1 @ v.T)                       [8, BPC]
            ps_h1 = psA.tile([RED, BPC], f32, tag="pa", name="ps_h1")
            nc.tensor.matmul(ps_h1[:], lhsT=caw1t_sb[:], rhs=vt_sb[:],
                             start=True, stop=True)
            h1t_sb = stage.tile([RED, BPC], f32)
            nc.scalar.activation(h1t_sb[:], ps_h1[:], AF.Prelu, alpha=0.1)

            # attT = sigmoid(ca_w2 @ h1)                     [64, BPC]
            ps_att = psA.tile([C, BPC], f32, tag="pa", name="ps_att")
            nc.tensor.matmul(ps_att[:], lhsT=caw2t_sb[:], rhs=h1t_sb[:],
                             start=True, stop=True)
            att_sb = stage.tile([C, BPC], f32)
            nc.scalar.activation(att_sb[:], ps_att[:], AF.Sigmoid)

            # h2T = lrelu(k_w1 @ v.T)                        [64, BPC]
            ps_h2 = psA.tile([C, BPC], f32, tag="pa", name="ps_h2")
            nc.tensor.matmul(ps_h2[:], lhsT=kw1t_sb[:], rhs=vt_sb[:],
                             start=True, stop=True)
            h2t_sb = stage.tile([C, BPC], f32)
            nc.scalar.activation(h2t_sb[:], ps_h2[:], AF.Prelu, alpha=0.1)

            # kern = h2 @ k_w2.T                             [BPC, 576]
            ps_k = psA.tile([BPC, NK], f32, tag="pa", name="ps_k")
            nc.tensor.matmul(ps_k[:, 0:512], lhsT=h2t_sb[:],
                             rhs=kw2t_sb[:, 0:512], start=True, stop=True)
            nc.tensor.matmul(ps_k[:, 512:NK], lhsT=h2t_sb[:],
                             rhs=kw2t_sb[:, 512:NK], start=True, stop=True)
            kern_sb = stage.tile([BPC, NK], f32)
            nc.scalar.activation(kern_sb[:], ps_k[:], AF.Copy)

            # ---- gather per-pair tap scalars: p = s*64 + c on partitions ----
            dtap_sb = stage.tile([128, PAIRS, KK * KK], f32)
            attpp_sb = stage.tile([128, PAIRS], f32)
            for pr in range(PAIRS):
                for s in range(2):
                    b = pr * 2 + s
                    # src [1, 64, 9] and dest [64, 1, 9] match in flattened
                    # element order (dma_start maps by flat AP order).
                    src = kern_sb[b:b + 1, :].rearrange(
                        "o (c t) -> o c t", c=C)
                    nc.sync.dma_start(
                        out=dtap_sb[C * s:C * (s + 1), pr:pr + 1, :], in_=src)
                    nc.sync.dma_start(
                        out=attpp_sb[C * s:C * (s + 1), pr:pr + 1],
                        in_=att_sb[:, b:b + 1])

            # tap split: even-dj taps run on the Vector engine (bf16 2x
            # mode needs 4-byte alignment); the rest run on the PE as
            # diagonal matmuls.
            DVE_TAPS = [(0, 0), (0, 2), (1, 0)]
            PE_TAPS = [(0, 1), (1, 1), (2, 1), (2, 0), (2, 2), (1, 2)]

            # d[p, t] = att[p] * kern[p, t]; diag tiles = eye * d[:, t]
            diag = [{} for _ in range(PAIRS)]
            dcols = []
            for pr in range(PAIRS):
                d_pr = stage.tile([128, KK * KK], f32, tag=f"d{pr}")
                nc.vector.tensor_scalar_mul(
                    d_pr[:], dtap_sb[:, pr, :], attpp_sb[:, pr:pr + 1])
                dcols.append(d_pr)  # noqa
                for (di, dj) in PE_TAPS:
                    t = di * KK + dj
                    dg = diags.tile([128, 128], bf16, tag=f"diag{pr}_{t}")
                    nc.vector.tensor_scalar_mul(
                        dg[:], eye_sb[:], d_pr[:, t:t + 1])
                    diag[pr][(di, dj)] = dg

            # ---- main loop ----
            xv = x.ap().rearrange("(pr s) c h w -> pr (s c) h w", pr=PAIRS)
            ov = out.ap().rearrange("(pr s) c h w -> pr (s c) h w", pr=PAIRS)

            GRP = 4  # chunks per weight-stationary group (psA banks)
            for pr in range(PAIRS):
                xt = xin.tile([128, HP, WP], bf16, tag="xt")
                # zero the one-pixel border; the interior is fully overwritten
                nc.vector.memset(xt[:, 0, :], 0.0)
                nc.vector.memset(xt[:, HP - 1, :], 0.0)
                nc.vector.memset(xt[:, 1:HP - 1, 0], 0.0)
                nc.vector.memset(xt[:, 1:HP - 1, WP - 1], 0.0)
                # split the 4 MiB load across DMA queues
                nsplit = 8
                rstep = H // nsplit
                for k in range(nsplit):
                    r0 = k * rstep
                    nc.sync.dma_start(
                        out=xt[:, 1 + r0:1 + r0 + rstep, 1:WP - 1],
                        in_=xv[pr, :, r0:r0 + rstep, :])

                BR = 2 * GRP * ROWS_PER_CHUNK  # partial rows per block (32)
                parts_of = {}
                for gb in range(0, NCHUNK, 2 * GRP):
                    r0 = (gb // (2 * GRP)) * BR
                    part = parts.tile([128, BR, W], bf16, tag="part",
                                      bufs=3, name=f"part{gb}")
                    parts_of[gb] = (part, 0)
                    parts_of[gb + GRP] = (part, GRP * ROWS_PER_CHUNK)
                    # Vector engine: accumulate the even-aligned taps for
                    # this 32-row block of the pair image (bf16 2x/4x modes)
                    # ts (4x mode) + tt-add (2x mode) beat one fused
                    # scalar_tensor_tensor (1x-only uop) on the DVE
                    for n, (di, dj) in enumerate(DVE_TAPS):
                        t = di * KK + dj
                        blk = part[:, :, :]
                        xin_v = xt[:, r0 + di:r0 + di + BR, dj:dj + W]
                        if n == 0:
                            nc.vector.tensor_scalar_mul(
                                blk, xin_v, dcols[pr][:, t:t + 1])
                        else:
                            tmp = parts.tile([128, BR, W], bf16, tag="tmp",
                                             bufs=3, name=f"tmp{gb}_{n}")
                            nc.vector.tensor_scalar_mul(
                                tmp[:], xin_v, dcols[pr][:, t:t + 1])
                            nc.vector.tensor_add(blk, blk, tmp[:])

                for g in range(0, NCHUNK, GRP):
                    part, roff = parts_of[g]
                    NW = ROWS_PER_CHUNK * W  # 512
                    pas = [psA.tile([128, 2 * NW], f32,
                                    tag="pa", name=f"pa{g}_{h}")
                           for h in range(GRP // 2)]
                    # PE: remaining taps as diagonal matmuls (weight loaded
                    # once per GRP), then the DVE partial via identity matmul
                    for t, (di, dj) in enumerate(PE_TAPS):
                        for c in range(GRP):
                            i0 = (g + c) * ROWS_PER_CHUNK
                            nc.tensor.matmul(
                                pas[c // 2][:, (c % 2) * NW:
                                            (c % 2) * NW + NW],
                                lhsT=diag[pr][(di, dj)][:],
                                rhs=xt[:, i0 + di:i0 + di + ROWS_PER_CHUNK,
                                       dj:dj + W],
                                start=(t == 0), stop=False,
                                skip_group_check=True)
                    for c in range(GRP):
                        rc = roff + c * ROWS_PER_CHUNK
                        nc.tensor.matmul(
                            pas[c // 2][:, (c % 2) * NW:(c % 2) * NW + NW],
                            lhsT=eyebf_sb[:],
                            rhs=part[:, rc:rc + ROWS_PER_CHUNK, :],
                            start=False, stop=True, skip_group_check=True)
                    for h in range(GRP // 2):
                        i0 = (g + 2 * h) * ROWS_PER_CHUNK
                        yt = ys.tile([128, 2 * NW], bf16, tag="yt")
                        nc.scalar.activation(yt[:], pas[h][:], AF.Prelu,
                                             alpha=0.1)
                        for c2 in range(2):
                            pb = psB.tile([128, NW], f32, tag="pb")
                            nc.tensor.matmul(
                                pb[:], lhsT=convt_sb[:],
                                rhs=yt[:, c2 * NW:c2 * NW + NW],
                                start=True, stop=True)
                            ot = osb.tile([128, NW], bf16, tag="ot")
                            nc.scalar.activation(ot[:], pb[:], AF.Identity,
                                                 bias=bcol_sb[:, 0:1])
                            j0 = i0 + c2 * ROWS_PER_CHUNK
                            nc.sync.dma_start(
                                out=ov[pr, :, j0:j0 + ROWS_PER_CHUNK, :],
                                in_=ot[:].rearrange("p (r w) -> p r w",
                                                    r=ROWS_PER_CHUNK))


def get_nc(repeat=1, n_taps=9):
    key = ("nc", repeat, n_taps)
    if key not in _CACHE:
        _CACHE[key] = _build(repeat, n_taps)
    return _CACHE[key]


def make_in_maps(x0, v, ca_w1, ca_w2, k_w1, k_w2, conv_w, conv_b):
    bf = ml_dtypes.bfloat16
    caw1t = np.ascontiguousarray(ca_w1.T, dtype=np.float32)
    caw2t = np.ascontiguousarray(ca_w2.T, dtype=np.float32)
    kw1t = np.ascontiguousarray(k_w1.T, dtype=np.float32)
    kw2t = np.ascontiguousarray(k_w2.T, dtype=np.float32)
    convt = np.zeros((128, 128), dtype=bf)
    cwt = conv_w.T.astype(bf)
    convt[0:64, 0:64] = cwt
    convt[64:128, 64:128] = cwt
    bcol = np.tile(conv_b.astype(np.float32), 2)[:, None].copy()
    eye = np.eye(128, dtype=np.float32)
    eyebf = np.eye(128, dtype=bf)
    in_maps = []
    for k in range(N_CORES):
        sl = slice(k * BPC, (k + 1) * BPC)
        in_maps.append({
            "x": np.ascontiguousarray(x0[sl]).astype(bf),
            "vt": np.ascontiguousarray(v[sl].T, dtype=np.float32),
            "caw1t": caw1t, "caw2t": caw2t, "kw1t": kw1t, "kw2t": kw2t,
            "convt": convt, "bcol": bcol, "eye": eye, "eyebf": eyebf,
        })
    return in_maps


def kernel(x0, v, ca_w1, ca_w2, k_w1, k_w2, conv_w, conv_b):
    from concourse.bass_utils import run_bass_kernel_spmd

    nc = get_nc()
    in_maps = make_in_maps(x0, v, ca_w1, ca_w2, k_w1, k_w2, conv_w, conv_b)
    res = run_bass_kernel_spmd(nc, in_maps, list(range(N_CORES)))
    return np.concatenate([res.results[i]["out"] for i in range(N_CORES)],
                          axis=0).astype(np.float32)


# revision 21
# speedup vs baseline: 1.1537x; 1.0933x over previous
"""Trainium2 Bass kernel for the per-sample dynamic-depthwise-conv block.

Computation (per sample b):
    att  = sigmoid(lrelu(v @ ca_w1.T) @ ca_w2.T)            # [b, 64]
    kern = (lrelu(v @ k_w1.T) @ k_w2.T).reshape(b*64,1,3,3) # per-(b,c) 3x3
    y    = lrelu(depthwise3x3(x0 * att, kern))
    out  = conv1x1(y, conv_w) + conv_b

Strategy: data-parallel over batch across 8 cores (4 samples/core).  On each
core, samples are processed in 2 "pairs"; a pair's 2x64 channels fill the 128
SBUF partitions.  The attention gate is folded into the generated tap weights
(dw(att*x) == att*dw(x) per channel), so x0 streams straight from HBM as bf16
into a zero-padded [128, 130, 130] SBUF tile.  The 9 depthwise taps are split
across engines:
  - 6 taps run on the TensorEngine as PSUM-accumulated matmuls with
    diagonal bf16 lhsT weights against shifted views of the padded tile
    (weight-stationary over groups of 4 chunks);
  - 3 even-aligned taps run on the Vector engine in bf16 (tensor_scalar at 4x
    mode + tensor_tensor add at 2x mode over 32-row blocks) and are injected
    into the same PSUM accumulation via one identity matmul per chunk.
Leaky-relu runs on the Scalar engine (Prelu, PSUM->SBUF, bf16); the final 1x1
conv is one matmul per 512-column chunk with a block-diagonal
[conv_w.T, conv_w.T] lhsT; conv bias rides the Scalar engine's Identity
activation during the PSUM->SBUF copy; outputs leave as bf16 and are widened
to fp32 on the host.  The tiny channel-attention/kernel-generating MLPs run
once per core on the PE/ACT engines at fp32.
"""

import sys

if "/opt/trn_rl_repo" not in sys.path:
    sys.path.append("/opt/trn_rl_repo")

import numpy as np
import ml_dtypes

B, C, H, W = 32, 64, 128, 128
KK = 3
RED = 8
N_CORES = 8
BPC = B // N_CORES          # samples per core (4)
PAIRS = BPC // 2            # sample pairs per core (2)
HP, WP = H + 2, W + 2       # zero-padded image dims (130)
ROWS_PER_CHUNK = 4          # output rows per matmul chunk -> N = 4*128 = 512
NCHUNK = H // ROWS_PER_CHUNK

_CACHE = {}


def _build(repeat=1, n_taps=9):
    import concourse.bass as bass  # noqa: F401
    from concourse import bacc, tile, mybir

    f32 = mybir.dt.float32
    bf16 = mybir.dt.bfloat16
    AF = mybir.ActivationFunctionType

    nc = bacc.Bacc(None, target_bir_lowering=False, debug=False)

    x = nc.dram_tensor("x", [BPC, C, H, W], bf16, kind="ExternalInput")
    vt = nc.dram_tensor("vt", [C, BPC], f32, kind="ExternalInput")
    caw1t = nc.dram_tensor("caw1t", [C, RED], f32, kind="ExternalInput")
    caw2t = nc.dram_tensor("caw2t", [RED, C], f32, kind="ExternalInput")
    kw1t = nc.dram_tensor("kw1t", [C, C], f32, kind="ExternalInput")
    kw2t = nc.dram_tensor("kw2t", [C, C * KK * KK], f32, kind="ExternalInput")
    convt = nc.dram_tensor("convt", [128, 128], bf16, kind="ExternalInput")
    bcol = nc.dram_tensor("bcol", [128, 1], f32, kind="ExternalInput")
    eye = nc.dram_tensor("eye", [128, 128], f32, kind="ExternalInput")
    eyebf = nc.dram_tensor("eyebf", [128, 128], bf16, kind="ExternalInput")
    out = nc.dram_tensor("out", [BPC, C, H, W], bf16, kind="ExternalOutput")

    NK = C * KK * KK  # 576

    with tile.TileContext(nc) as tc:
        with (
            tc.tile_pool(name="consts", bufs=1) as consts,
            tc.tile_pool(name="stage", bufs=1) as stage,
            tc.tile_pool(name="diags", bufs=1) as diags,
            tc.tile_pool(name="xin", bufs=2) as xin,
            tc.tile_pool(name="parts", bufs=2) as parts,
            tc.tile_pool(name="ys", bufs=5) as ys,
            tc.tile_pool(name="os", bufs=6) as osb,
            tc.tile_pool(name="psA", bufs=3, space="PSUM") as psA,
            tc.tile_pool(name="psB", bufs=2, space="PSUM") as psB,
        ):
            # ---- constants into SBUF ----
            vt_sb = consts.tile([C, BPC], f32)
            caw1t_sb = consts.tile([C, RED], f32)
            caw2t_sb = consts.tile([RED, C], f32)
            kw1t_sb = consts.tile([C, C], f32)
            kw2t_sb = consts.tile([C, NK], f32)
            convt_sb = consts.tile([128, 128], bf16)
            bcol_sb = consts.tile([128, 1], f32)
            eye_sb = consts.tile([128, 128], f32)
            eyebf_sb = consts.tile([128, 128], bf16)
            for t, d in (
                (vt_sb, vt), (caw1t_sb, caw1t), (caw2t_sb, caw2t),
                (kw1t_sb, kw1t), (kw2t_sb, kw2t), (convt_sb, convt),
                (bcol_sb, bcol), (eye_sb, eye), (eyebf_sb, eyebf),
            ):
                nc.sync.dma_start(out=t[:], in_=d.ap())

            import contextlib
            rep_ctx = (tc.For_i(0, repeat, 1) if repeat > 1
                       else contextlib.nullcontext())
            with rep_ctx:
                _body(nc, tc, mybir, AF, f32, bf16,
                      consts, stage, diags, xin, parts, ys, osb,
                      psA, psB, psA,
                      vt_sb, caw1t_sb, caw2t_sb, kw1t_sb, kw2t_sb,
                      convt_sb, bcol_sb, eye_sb, eyebf_sb, x, out, n_taps)

    nc.compile()
    return nc


def _body(nc, tc, mybir, AF, f32, bf16,
          consts, stage, diags, xin, parts, ys, osb, psA, psB, psS,
          vt_sb, caw1t_sb, caw2t_sb, kw1t_sb, kw2t_sb,
          convt_sb, bcol_sb, eye_sb, eyebf_sb, x, out, n_taps=9):
    NK = C * KK * KK
    if True:
        if True:

            # ---- tiny MLP stage: attention + generated kernels ----
            # h1T = lrelu(ca_w1 @ v.T)                       [8, BPC]
            ps_h1 = psA.tile([RED, BPC], f32, tag="pa", name="ps_h1")
            nc.tensor.matmul(ps_h1[:], lhsT=caw1t_sb[:], rhs=vt_sb[:],
                             start=True, stop=True)
            h1t_sb = stage.tile([RED, BPC], f32)
            nc.scalar.activation(h1t_sb[:], ps_h1[:], AF.Prelu, alpha=0.1)

            # attT = sigmoid(ca_w2 @ h1)                     [64, BPC]
            ps_att = psA.tile([C, BPC], f32, tag="pa", name="ps_att")
            nc.tensor.matmul(ps_att[:], lhsT=caw2t_sb[:], rhs=h1t_sb[:],
                             start=True, stop=True)
            att_sb = stage.tile([C, BPC], f32)
            nc.scalar.activation(att_sb[:], ps_att[:], AF.Sigmoid)

            # h2T = lrelu(k_w1 @ v.T)                        [64, BPC]
            ps_h2 = psA.tile([C, BPC], f32, tag="pa", name="ps_h2")
            nc.tensor.matmul(ps_h2[:], lhsT=kw1t_sb[:], rhs=vt_sb[:],
                             start=True, stop=True)
            h2t_sb = stage.tile([C, BPC], f32)
            nc.scalar.activation(h2t_sb[:], ps_h2[:], AF.Prelu, alpha=0.1)

            # kern = h2 @ k_w2.T                             [BPC, 576]
            ps_k = psA.tile([BPC, NK], f32, tag="pa", name="ps_k")
            nc.tensor.matmul(ps_k[:, 0:512], lhsT=h2t_sb[:],
                             rhs=kw2t_sb[:, 0:512], start=True, stop=True)
            nc.tensor.matmul(ps_k[:, 512:NK], lhsT=h2t_sb[:],
                             rhs=kw2t_sb[:, 512:NK], start=True, stop=True)
            kern_sb = stage.tile([BPC, NK], f32)
            nc.scalar.activation(kern_sb[:], ps_k[:], AF.Copy)

            # ---- gather per-pair tap scalars: p = s*64 + c on partitions ----
            dtap_sb = stage.tile([128, PAIRS, KK * KK], f32)
            attpp_sb = stage.tile([128, PAIRS], f32)
            for pr in range(PAIRS):
                for s in range(2):
                    b = pr * 2 + s
                    # src [1, 64, 9] and dest [64, 1, 9] match in flattened
                    # element order (dma_start maps by flat AP order).
                    src = kern_sb[b:b + 1, :].rearrange(
                        "o (c t) -> o c t", c=C)
                    nc.sync.dma_start(
                        out=dtap_sb[C * s:C * (s + 1), pr:pr + 1, :], in_=src)
                    nc.sync.dma_start(
                        out=attpp_sb[C * s:C * (s + 1), pr:pr + 1],
                        in_=att_sb[:, b:b + 1])

            # tap split: even-dj taps run on the Vector engine (bf16 2x
            # mode needs 4-byte alignment); the rest run on the PE as
            # diagonal matmuls.
            DVE_TAPS = [(0, 0), (0, 2), (1, 0)]
            PE_TAPS = [(0, 1), (1, 1), (2, 1), (2, 0), (2, 2), (1, 2)]

            # d[p, t] = att[p] * kern[p, t]; diag tiles = eye * d[:, t]
            diag = [{} for _ in range(PAIRS)]
            dcols = []
            for pr in range(PAIRS):
                d_pr = stage.tile([128, KK * KK], f32, tag=f"d{pr}")
                nc.vector.tensor_scalar_mul(
                    d_pr[:], dtap_sb[:, pr, :], attpp_sb[:, pr:pr + 1])
                dcols.append(d_pr)  # noqa
                for (di, dj) in PE_TAPS:
                    t = di * KK + dj
                    dg = diags.tile([128, 128], bf16, tag=f"diag{pr}_{t}")
                    nc.vector.tensor_scalar_mul(
                        dg[:], eye_sb[:], d_pr[:, t:t + 1])
                    diag[pr][(di, dj)] = dg

            # ---- main loop ----
            xv = x.ap().rearrange("(pr s) c h w -> pr (s c) h w", pr=PAIRS)
            ov = out.ap().rearrange("(pr s) c h w -> pr (s c) h w", pr=PAIRS)

            GRP = 4  # chunks per weight-stationary group (psA banks)
            for pr in range(PAIRS):
                xt = xin.tile([128, HP, WP], bf16, tag="xt")
                # zero the one-pixel border; the interior is fully overwritten
                nc.vector.memset(xt[:, 0, :], 0.0)
                nc.vector.memset(xt[:, HP - 1, :], 0.0)
                nc.vector.memset(xt[:, 1:HP - 1, 0], 0.0)
                nc.vector.memset(xt[:, 1:HP - 1, WP - 1], 0.0)
                # split the 4 MiB load across DMA queues
                nsplit = 8
                rstep = H // nsplit
                for k in range(nsplit):
                    r0 = k * rstep
                    nc.sync.dma_start(
                        out=xt[:, 1 + r0:1 + r0 + rstep, 1:WP - 1],
                        in_=xv[pr, :, r0:r0 + rstep, :])

                BR = 2 * GRP * ROWS_PER_CHUNK  # partial rows per block (32)
                parts_of = {}
                for gb in range(0, NCHUNK, 2 * GRP):
                    r0 = (gb // (2 * GRP)) * BR
                    part = parts.tile([128, BR, W], bf16, tag="part",
                                      bufs=3, name=f"part{gb}")
                    parts_of[gb] = (part, 0)
                    parts_of[gb + GRP] = (part, GRP * ROWS_PER_CHUNK)
                    # Vector engine: accumulate the even-aligned taps for
                    # this 32-row block of the pair image (bf16 2x/4x modes)
                    # ts (4x mode) + tt-add (2x mode) beat one fused
                    # scalar_tensor_tensor (1x-only uop) on the DVE
                    for n, (di, dj) in enumerate(DVE_TAPS):
                        t = di * KK + dj
                        blk = part[:, :, :]
                        xin_v = xt[:, r0 + di:r0 + di + BR, dj:dj + W]
                        if n == 0:
                            nc.vector.tensor_scalar_mul(
                                blk, xin_v, dcols[pr][:, t:t + 1])
                        else:
                            tmp = parts.tile([128, BR, W], bf16, tag="tmp",
                                             bufs=3, name=f"tmp{gb}_{n}")
                            nc.vector.tensor_scalar_mul(
                                tmp[:], xin_v, dcols[pr][:, t:t + 1])
                            nc.vector.tensor_add(blk, blk, tmp[:])

                for g in range(0, NCHUNK, GRP):
                    part, roff = parts_of[g]
                    NW = ROWS_PER_CHUNK * W  # 512
                    pas = [psA.tile([128, 2 * NW], f32,
                                    tag="pa", name=f"pa{g}_{h}")
                           for h in range(GRP // 2)]
                    # PE: remaining taps as diagonal matmuls (weight loaded
                    # once per GRP), then the DVE partial via identity matmul
                    for t, (di, dj) in enumerate(PE_TAPS):
                        for c in range(GRP):
                            i0 = (g + c) * ROWS_PER_CHUNK
                            nc.tensor.matmul(
                                pas[c // 2][:, (c % 2) * NW:
                                            (c % 2) * NW + NW],
                                lhsT=diag[pr][(di, dj)][:],
                                rhs=xt[:, i0 + di:i0 + di + ROWS_PER_CHUNK,
                                       dj:dj + W],
                                start=(t == 0), stop=False,
                                skip_group_check=True)
                    for c in range(GRP):
                        rc = roff + c * ROWS_PER_CHUNK
                        nc.tensor.matmul(
                            pas[c // 2][:, (c % 2) * NW:(c % 2) * NW + NW],
                            lhsT=eyebf_sb[:],
                            rhs=part[:, rc:rc + ROWS_PER_CHUNK, :],
                            start=False, stop=True, skip_group_check=True)
                    for h in range(GRP // 2):
                        i0 = (g + 2 * h) * ROWS_PER_CHUNK
                        yt = ys.tile([128, 2 * NW], bf16, tag="yt")
                        nc.scalar.activation(yt[:], pas[h][:], AF.Prelu,
                                             alpha=0.1)
                        for c2 in range(2):
                            pb = psB.tile([128, NW], f32, tag="pb")
                            nc.tensor.matmul(
                                pb[:], lhsT=convt_sb[:],
                                rhs=yt[:, c2 * NW:c2 * NW + NW],
                                start=True, stop=True)
                            ot = osb.tile([128, NW], bf16, tag="ot")
                            nc.scalar.activation(ot[:], pb[:], AF.Identity,
                                                 bias=bcol_sb[:, 0:1])
                            j0 = i0 + c2 * ROWS_PER_CHUNK
                            nc.sync.dma_start(
                                out=ov[pr, :, j0:j0 + ROWS_PER_CHUNK, :],
                                in_=ot[:].rearrange("p (r w) -> p r w",
                                                    r=ROWS_PER_CHUNK))


def get_nc(repeat=1, n_taps=9):
    key = ("nc", repeat, n_taps)
    if key not in _CACHE:
        _CACHE[key] = _build(repeat, n_taps)
    return _CACHE[key]


def make_in_maps(x0, v, ca_w1, ca_w2, k_w1, k_w2, conv_w, conv_b):
    bf = ml_dtypes.bfloat16
    caw1t = np.ascontiguousarray(ca_w1.T, dtype=np.float32)
    caw2t = np.ascontiguousarray(ca_w2.T, dtype=np.float32)
    kw1t = np.ascontiguousarray(k_w1.T, dtype=np.float32)
    kw2t = np.ascontiguousarray(k_w2.T, dtype=np.float32)
    convt = np.zeros((128, 128), dtype=bf)
    cwt = conv_w.T.astype(bf)
    convt[0:64, 0:64] = cwt
    convt[64:128, 64:128] = cwt
    bcol = np.tile(conv_b.astype(np.float32), 2)[:, None].copy()
    eye = np.eye(128, dtype=np.float32)
    eyebf = np.eye(128, dtype=bf)
    in_maps = []
    for k in range(N_CORES):
        sl = slice(k * BPC, (k + 1) * BPC)
        in_maps.append({
            "x": np.ascontiguousarray(x0[sl]).astype(bf),
            "vt": np.ascontiguousarray(v[sl].T, dtype=np.float32),
            "caw1t": caw1t, "caw2t": caw2t, "kw1t": kw1t, "kw2t": kw2t,
            "convt": convt, "bcol": bcol, "eye": eye, "eyebf": eyebf,
        })
    return in_maps


def kernel(x0, v, ca_w1, ca_w2, k_w1, k_w2, conv_w, conv_b):
    from concourse.bass_utils import run_bass_kernel_spmd

    nc = get_nc()
    in_maps = make_in_maps(x0, v, ca_w1, ca_w2, k_w1, k_w2, conv_w, conv_b)
    res = run_bass_kernel_spmd(nc, in_maps, list(range(N_CORES)))
    return np.concatenate([res.results[i]["out"] for i in range(N_CORES)],
                          axis=0).astype(np.float32)
